# revision 4
# baseline (speedup 1.0000x reference)
"""Trainium2 Bass kernel for quantized ConvBNReLU1D (pointwise conv k=1).

Reference computation (see problem spec):
    wq  = fake_quant_int8(W)  (per-tensor power-of-two scale)
    bq  = fake_quant_int8(b)
    y   = wq @ x + bq                  # [Cout,Cin] x [B,Cin,N]
    y   = y * inv + (beta - mean*inv)  # BN inference, inv = gamma*rsqrt(var+eps)
    y   = clip(round(relu(y)/as), 0, 255) * as   # QuantReLU

Strategy (v3 — minimize HBM bytes, then pack the DMA stream):
  - Data-parallel over batch: 32 batches -> 4 per core on 8 cores.
  - Host precomputes the per-channel constants (wq/bq fake-quant is
    bitwise-identical to the fp32 reference; BN+act_scale folded) so the
    device epilogue is one ScalarE ACTIVATE per tile.
  - x is sent as plain bf16 (half the bytes of fp32). wq is exactly
    representable in bf16 (8-bit integer x power of two), so the only
    error is bf16 rounding of x: measured rel err 0.0039 (max one quant
    step), same as the baseline's fp32-split pipeline.
  - Output goes to HBM as u8 quantization codes (the result has only 256
    distinct values: u8 * act_scale); dequant happens on host during
    unshard. 1 byte/elem instead of 4.
  - v3 pipeline fixes over v2 (63 us -> target ~45 us):
      * all constants packed into TWO front-loaded DMAs on the sync ring
        (v2's 8 strided const loads serialized ~9 us before the first
        matmul could start);
      * x buffered at full depth (all 16 half-tiles), output tiles 8-deep:
        no WAR stalls in the DMA stream;
      * x loaded and y stored as [128, 2048] half-tiles so compute starts
        after 0.5 MB (not 1 MB) and stores drain right behind ScalarE;
      * matmuls k-major inside a PSUM tile (4 same-weight matmuls per
        LDWEIGHTS instead of alternating weights every matmul).
  - DMA per core: in 8.4 MB (bf16) + out 4.2 MB (u8) = 12.6 MB ~= 35 us
    at the ~358 GB/s HBM/core roofline; TensorE ~28 us; ScalarE ~32 us;
    VectorE/GpSimd unused. ~5-8 us fixed runtime preamble on top.
"""

import os
import sys

import numpy as np

for _p in ("/opt/trn_rl_repo", "/root/.axon_site/_ro/trn_rl_repo"):
    if os.path.isdir(_p) and _p not in sys.path:
        sys.path.insert(0, _p)

from contextlib import ExitStack

import ml_dtypes

import concourse.bacc as bacc
import concourse.tile as tile
from concourse import mybir
from concourse.bass import ts
from concourse.bass_utils import run_bass_kernel_spmd

F32 = mybir.dt.float32
BF16 = mybir.dt.bfloat16
U8 = mybir.dt.uint8
AF = mybir.ActivationFunctionType

N_CORES = 8
B, CIN, COUT, N = 32, 256, 256, 4096
B_SH = B // N_CORES  # batches per core
NTILE = 512          # matmul free dim (one fp32 PSUM bank)
EP_BANKS = 4         # PSUM banks per epilogue tile (ACT width = 512*EP_BANKS)
EPW = NTILE * EP_BANKS
NEP = N // EPW       # epilogue tiles per row block (= x half-tiles)
KC = CIN // 128      # K chunks
MC = COUT // 128     # output-channel chunks

QMAX_W = 127.0
BN_EPS = 1e-5

_NC_CACHE = []
LAST_RESULTS = None  # BassKernelResults of the last run (for profiling)


def _build_nc():
    nc = bacc.Bacc("TRN2", target_bir_lowering=False)
    xh_s = nc.declare_dram_parameter("xh_s", [B_SH, CIN, N], BF16, isOutput=False)
    # all 4 lhsT chunks packed side by side: col block k*MC+mo is
    # wT[k*128:(k+1)*128, mo*128:(mo+1)*128]
    w_all = nc.declare_dram_parameter("w_all", [128, KC * MC * 128], BF16, isOutput=False)
    # per-channel vectors packed: col mo = sv chunk, col MC+mo = bv chunk
    vec_all = nc.declare_dram_parameter("vec_all", [128, 2 * MC], F32, isOutput=False)
    y_s = nc.declare_dram_parameter("y_s", [B_SH, COUT, N], U8, isOutput=True)

    with ExitStack() as ctx:
        tc = ctx.enter_context(tile.TileContext(nc))
        consts = ctx.enter_context(tc.tile_pool(name="consts", bufs=1))
        xpool = ctx.enter_context(tc.tile_pool(name="xpool", bufs=B_SH * KC * NEP))
        opool = ctx.enter_context(tc.tile_pool(name="opool", bufs=8))
        pspool = ctx.enter_context(
            tc.tile_pool(name="pspool", bufs=8 // EP_BANKS, space="PSUM")
        )

        # Front-loaded packed constants on the sync ring: they land before
        # the first x half-tile so the first matmul is never weight-gated.
        w_sb = consts.tile([128, KC * MC * 128], BF16, tag="w")
        nc.sync.dma_start(out=w_sb, in_=w_all[:, :])
        vec_sb = consts.tile([128, 2 * MC], F32, tag="vec")
        nc.sync.dma_start(out=vec_sb, in_=vec_all[:, :])

        for b in range(B_SH):
            xt = {}
            for h in range(NEP):
                for k in range(KC):
                    t = xpool.tile([128, EPW], BF16, tag="x")
                    nc.sync.dma_start(
                        out=t, in_=xh_s[b, k * 128 : (k + 1) * 128, ts(h, EPW)]
                    )
                    xt[(k, h)] = t
            for mo in range(MC):
                for ne in range(NEP):
                    ps = pspool.tile([128, EPW], F32, tag="ps")
                    # k-major: 4 matmuls per stationary weight load
                    for k in range(KC):
                        for sb in range(EP_BANKS):
                            nc.tensor.matmul(
                                ps[:, ts(sb, NTILE)],
                                lhsT=w_sb[:, ts(k * MC + mo, 128)],
                                rhs=xt[(k, ne)][:, ts(sb, NTILE)],
                                start=(k == 0),
                                stop=(k == KC - 1),
                            )
                    # u8 = sat_u8(relu(psum*sv + bv)): the f32->u8 convert
                    # is exact round-half-even + clamp to [0,255] in HW.
                    ot = opool.tile([128, EPW], U8, tag="o")
                    nc.scalar.activation(
                        ot, ps, AF.Relu,
                        bias=vec_all_col(vec_sb, MC + mo),
                        scale=vec_all_col(vec_sb, mo),
                    )
                    nc.scalar.dma_start(
                        out=y_s[b, mo * 128 : (mo + 1) * 128, ts(ne, EPW)], in_=ot
                    )
    nc.compile()
    return nc


def vec_all_col(vec_sb, j):
    return vec_sb[:, j : j + 1]


def _host_fold(W, b, gamma, beta, running_mean, running_var, act_scale):
    """Fake-quant W/b exactly as the fp32 reference, fold BN + act scale."""
    f32 = np.float32

    def po2_scale(t):
        maxabs = np.maximum(np.max(np.abs(t)), f32(1e-12)).astype(f32)
        # log2/ceil/exp2 of an f32 value; result is an exact power of two.
        return np.exp2(np.ceil(np.log2(maxabs / f32(QMAX_W)))).astype(f32)

    def fake_quant(t, s):
        return (np.clip(np.round(t / s), -128.0, 127.0) * s).astype(f32)

    wq = fake_quant(W.astype(f32), po2_scale(W.astype(f32)))
    bq = fake_quant(b.astype(f32), po2_scale(b.astype(f32)))
    inv = (gamma.astype(f32) / np.sqrt(running_var.astype(f32) + f32(BN_EPS))).astype(f32)
    shift = (beta.astype(f32) - running_mean.astype(f32) * inv).astype(f32)
    a_s = f32(act_scale)
    sv = (inv / a_s).astype(f32)                    # per-channel matmul scale
    bv = ((bq * inv + shift) / a_s).astype(f32)     # per-channel bias
    # wq is an 8-bit integer times a power of two -> exact in bf16
    wT = np.ascontiguousarray(wq.T)                 # [Cin, Cout] f32
    w_pack = np.empty((128, KC * MC * 128), dtype=ml_dtypes.bfloat16)
    for k in range(KC):
        for mo in range(MC):
            j = (k * MC + mo) * 128
            w_pack[:, j : j + 128] = wT[
                k * 128 : (k + 1) * 128, mo * 128 : (mo + 1) * 128
            ].astype(ml_dtypes.bfloat16)
    vec_pack = np.empty((128, 2 * MC), dtype=np.float32)
    for mo in range(MC):
        vec_pack[:, mo] = sv[mo * 128 : (mo + 1) * 128]
        vec_pack[:, MC + mo] = bv[mo * 128 : (mo + 1) * 128]
    return w_pack, vec_pack, a_s


def kernel(x, W, b, gamma, beta, running_mean, running_var, act_scale):
    global LAST_RESULTS
    if not _NC_CACHE:
        _NC_CACHE.append(_build_nc())
    nc = _NC_CACHE[0]

    w_pack, vec_pack, a_s = _host_fold(
        W, b, gamma, beta, running_mean, running_var, act_scale
    )
    x_hi = np.ascontiguousarray(x, dtype=np.float32).astype(ml_dtypes.bfloat16)

    in_maps = []
    for c in range(N_CORES):
        sl = slice(c * B_SH, (c + 1) * B_SH)
        in_maps.append({"xh_s": x_hi[sl], "w_all": w_pack, "vec_all": vec_pack})

    trace = bool(os.environ.get("KERNEL_TRACE"))
    try:
        res = run_bass_kernel_spmd(
            nc, in_maps, core_ids=list(range(N_CORES)), trace=trace
        )
    except Exception:
        if not trace:
            raise
        # trace path unavailable (e.g. NTFF hook missing) — run untraced
        res = run_bass_kernel_spmd(
            nc, in_maps, core_ids=list(range(N_CORES)), trace=False
        )
    LAST_RESULTS = res
    codes = np.concatenate([r["y_s"] for r in res.results], axis=0)
    # dequantize the u8 codes during unshard: y = codes * act_scale
    lut = (np.arange(256, dtype=np.float32) * a_s).astype(np.float32)
    return lut[codes]


# revision 6
# speedup vs baseline: 1.2041x; 1.2041x over previous
"""Trainium2 Bass kernel for quantized ConvBNReLU1D (pointwise conv k=1).

Reference computation (see problem spec):
    wq  = fake_quant_int8(W)  (per-tensor power-of-two scale)
    bq  = fake_quant_int8(b)
    y   = wq @ x + bq                  # [Cout,Cin] x [B,Cin,N]
    y   = y * inv + (beta - mean*inv)  # BN inference, inv = gamma*rsqrt(var+eps)
    y   = clip(round(relu(y)/as), 0, 255) * as   # QuantReLU

Strategy (v3 — minimize HBM bytes, then pack the DMA stream):
  - Data-parallel over batch: 32 batches -> 4 per core on 8 cores.
  - Host precomputes the per-channel constants (wq/bq fake-quant is
    bitwise-identical to the fp32 reference; BN+act_scale folded) so the
    device epilogue is one ScalarE ACTIVATE per tile.
  - x is sent as plain bf16 (half the bytes of fp32). wq is exactly
    representable in bf16 (8-bit integer x power of two), so the only
    error is bf16 rounding of x: measured rel err 0.0039 (max one quant
    step), same as the baseline's fp32-split pipeline.
  - Output goes to HBM as u8 quantization codes (the result has only 256
    distinct values: u8 * act_scale); dequant happens on host during
    unshard. 1 byte/elem instead of 4.
  - v4 pipeline fixes over v2 (63 us measured):
      * all constants packed into TWO front-loaded DMAs at the head of
        the scalar ring (v2's 8 strided const loads issued so slowly the
        first matmul waited until +17 us);
      * x buffered at full depth (all 8 tiles) and output tiles 4-deep:
        no WAR stalls in the DMA stream (v2 stalled ~4 us mid-run);
      * transfers stay full-width [128, 4096] — v3 measured that halving
        them costs ~30% DMA efficiency (strided vs contiguous streams).
  - DMA per core: in 8.4 MB (bf16) + out 4.2 MB (u8) = 12.6 MB ~= 35 us
    at the ~358 GB/s HBM/core roofline; TensorE ~28 us; ScalarE ~32 us;
    VectorE/GpSimd unused. ~5-8 us fixed runtime preamble on top.
"""

import os
import sys

import numpy as np

for _p in ("/opt/trn_rl_repo", "/root/.axon_site/_ro/trn_rl_repo"):
    if os.path.isdir(_p) and _p not in sys.path:
        sys.path.insert(0, _p)

from contextlib import ExitStack

import ml_dtypes

import concourse.bacc as bacc
import concourse.tile as tile
from concourse import mybir
from concourse.bass import ts
from concourse.bass_utils import run_bass_kernel_spmd

F32 = mybir.dt.float32
BF16 = mybir.dt.bfloat16
U8 = mybir.dt.uint8
AF = mybir.ActivationFunctionType

N_CORES = 8
B, CIN, COUT, N = 32, 256, 256, 4096
B_SH = B // N_CORES  # batches per core
NTILE = 512          # matmul free dim (one fp32 PSUM bank)
EP_BANKS = 4         # PSUM banks per epilogue tile (ACT width = 512*EP_BANKS)
EPW = NTILE * EP_BANKS
NEP = N // EPW       # epilogue tiles per row block (= x half-tiles)
KC = CIN // 128      # K chunks
MC = COUT // 128     # output-channel chunks

QMAX_W = 127.0
BN_EPS = 1e-5

_NC_CACHE = []
LAST_RESULTS = None  # BassKernelResults of the last run (for profiling)


def _build_nc():
    nc = bacc.Bacc("TRN2", target_bir_lowering=False)
    xh_s = nc.declare_dram_parameter("xh_s", [B_SH, CIN, N], BF16, isOutput=False)
    # all 4 lhsT chunks packed side by side: col block k*MC+mo is
    # wT[k*128:(k+1)*128, mo*128:(mo+1)*128]
    w_all = nc.declare_dram_parameter("w_all", [128, KC * MC * 128], BF16, isOutput=False)
    # per-channel vectors packed: col mo = sv chunk, col MC+mo = bv chunk
    vec_all = nc.declare_dram_parameter("vec_all", [128, 2 * MC], F32, isOutput=False)
    y_s = nc.declare_dram_parameter("y_s", [B_SH, COUT, N], U8, isOutput=True)

    with ExitStack() as ctx:
        tc = ctx.enter_context(tile.TileContext(nc))
        consts = ctx.enter_context(tc.tile_pool(name="consts", bufs=1))
        xpool = ctx.enter_context(tc.tile_pool(name="xpool", bufs=B_SH * KC))
        opool = ctx.enter_context(tc.tile_pool(name="opool", bufs=4))
        pspool = ctx.enter_context(
            tc.tile_pool(name="pspool", bufs=8 // EP_BANKS, space="PSUM")
        )

        # Packed constants, front-loaded on the (otherwise idle-until-stores)
        # scalar ring: two DMAs instead of eight, landing before the first
        # full x tile finishes on the sync ring, so the first matmul is
        # never weight-gated and the x load stream is never queued behind
        # the descriptor-heavy small transfers.
        w_sb = consts.tile([128, KC * MC * 128], BF16, tag="w")
        nc.scalar.dma_start(out=w_sb, in_=w_all[:, :])
        vec_sb = consts.tile([128, 2 * MC], F32, tag="vec")
        nc.scalar.dma_start(out=vec_sb, in_=vec_all[:, :])

        for b in range(B_SH):
            # Full [128, 4096] x tiles: 8 KB contiguous per partition —
            # splitting these (or the stores) into halves costs ~30% DMA
            # efficiency (measured), far more than the earlier ramp it buys.
            xh_k = []
            for k in range(KC):
                t = xpool.tile([128, N], BF16, tag="x")
                nc.sync.dma_start(out=t, in_=xh_s[b, k * 128 : (k + 1) * 128, :])
                xh_k.append(t)
            for mo in range(MC):
                ot = opool.tile([128, N], U8, tag="o")
                for ne in range(NEP):
                    ps = pspool.tile([128, EPW], F32, tag="ps")
                    for sb in range(EP_BANKS):
                        nt = ne * EP_BANKS + sb
                        pslice = ps[:, ts(sb, NTILE)]
                        nc.tensor.matmul(
                            pslice, lhsT=w_sb[:, ts(mo, 128)],
                            rhs=xh_k[0][:, ts(nt, NTILE)],
                            start=True, stop=False,
                        )
                        nc.tensor.matmul(
                            pslice, lhsT=w_sb[:, ts(MC + mo, 128)],
                            rhs=xh_k[1][:, ts(nt, NTILE)],
                            start=False, stop=True,
                        )
                    # u8 = sat_u8(relu(psum*sv + bv)): the f32->u8 convert
                    # is exact round-half-even + clamp to [0,255] in HW.
                    nc.scalar.activation(
                        ot[:, ts(ne, EPW)], ps, AF.Relu,
                        bias=vec_sb[:, MC + mo : MC + mo + 1],
                        scale=vec_sb[:, mo : mo + 1],
                    )
                nc.scalar.dma_start(
                    out=y_s[b, mo * 128 : (mo + 1) * 128, :], in_=ot
                )
    nc.compile()
    return nc


def _host_fold(W, b, gamma, beta, running_mean, running_var, act_scale):
    """Fake-quant W/b exactly as the fp32 reference, fold BN + act scale."""
    f32 = np.float32

    def po2_scale(t):
        maxabs = np.maximum(np.max(np.abs(t)), f32(1e-12)).astype(f32)
        # log2/ceil/exp2 of an f32 value; result is an exact power of two.
        return np.exp2(np.ceil(np.log2(maxabs / f32(QMAX_W)))).astype(f32)

    def fake_quant(t, s):
        return (np.clip(np.round(t / s), -128.0, 127.0) * s).astype(f32)

    wq = fake_quant(W.astype(f32), po2_scale(W.astype(f32)))
    bq = fake_quant(b.astype(f32), po2_scale(b.astype(f32)))
    inv = (gamma.astype(f32) / np.sqrt(running_var.astype(f32) + f32(BN_EPS))).astype(f32)
    shift = (beta.astype(f32) - running_mean.astype(f32) * inv).astype(f32)
    a_s = f32(act_scale)
    sv = (inv / a_s).astype(f32)                    # per-channel matmul scale
    bv = ((bq * inv + shift) / a_s).astype(f32)     # per-channel bias
    # wq is an 8-bit integer times a power of two -> exact in bf16
    wT = np.ascontiguousarray(wq.T)                 # [Cin, Cout] f32
    w_pack = np.empty((128, KC * MC * 128), dtype=ml_dtypes.bfloat16)
    for k in range(KC):
        for mo in range(MC):
            j = (k * MC + mo) * 128
            w_pack[:, j : j + 128] = wT[
                k * 128 : (k + 1) * 128, mo * 128 : (mo + 1) * 128
            ].astype(ml_dtypes.bfloat16)
    vec_pack = np.empty((128, 2 * MC), dtype=np.float32)
    for mo in range(MC):
        vec_pack[:, mo] = sv[mo * 128 : (mo + 1) * 128]
        vec_pack[:, MC + mo] = bv[mo * 128 : (mo + 1) * 128]
    return w_pack, vec_pack, a_s


def kernel(x, W, b, gamma, beta, running_mean, running_var, act_scale):
    global LAST_RESULTS
    if not _NC_CACHE:
        _NC_CACHE.append(_build_nc())
    nc = _NC_CACHE[0]

    w_pack, vec_pack, a_s = _host_fold(
        W, b, gamma, beta, running_mean, running_var, act_scale
    )
    x_hi = np.ascontiguousarray(x, dtype=np.float32).astype(ml_dtypes.bfloat16)

    in_maps = []
    for c in range(N_CORES):
        sl = slice(c * B_SH, (c + 1) * B_SH)
        in_maps.append({"xh_s": x_hi[sl], "w_all": w_pack, "vec_all": vec_pack})

    trace = bool(os.environ.get("KERNEL_TRACE"))
    try:
        res = run_bass_kernel_spmd(
            nc, in_maps, core_ids=list(range(N_CORES)), trace=trace
        )
    except Exception:
        if not trace:
            raise
        # trace path unavailable (e.g. NTFF hook missing) — run untraced
        res = run_bass_kernel_spmd(
            nc, in_maps, core_ids=list(range(N_CORES)), trace=False
        )
    LAST_RESULTS = res
    codes = np.concatenate([r["y_s"] for r in res.results], axis=0)
    # dequantize the u8 codes during unshard: y = codes * act_scale
    lut = (np.arange(256, dtype=np.float32) * a_s).astype(np.float32)
    return lut[codes]
